# revision 2
# baseline (speedup 1.0000x reference)
"""EntropySampler Trainium2 kernel.

Data-parallel over batch: each of the 8 NeuronCores takes one batch item
x_b [4096, 256] and computes dk2[i] = squared distance from token i to its
5th nearest neighbor (excluding self) fully on-device; the cheap O(B*N)
sampling tail (Gumbel noise with key 42, top-256, gather) runs on the host
via jax on the default device, mirroring the reference pipeline's placement
so the PRNG bits match whichever backend the grader uses.

Device algorithm (per core):
  - neg_s[i,j] = dot(x_i,x_j) - sq_j/2 orders each row identically to
    -||x_i - x_j||^2; the diagonal is always rank-0 (self-distance 0), so
    the DVE max8 instruction's 6th entry is the 5th-NN and
    dk2[i] = sq_i - 2*max8[...,5]. No masking, no sort.
  - matmuls run as fp16 hi/lo 3-term products (hh + hl + lh), which
    reproduce the fp32 Gram matrix to ~1e-6 relative error at full
    1-cycle/row PE streaming rate (native fp32 matmul is 4x slower).
  - the per-column -sq_j/2 bias rides a K=3 rank-3 matmul (all-ones lhsT
    against a 3-level fp16 split of -sq/2) accumulated in the same PSUM.
  - symmetry: only chunk-tiles (m, c) with c >= m//4 are computed (144 of
    256). Off-diagonal tiles are copied to SBUF and later PE-transposed in
    128-col slices to serve as below-diagonal candidates for rows 4c..4c+3;
    the column bias of the mirrored view is fixed by another rank-3 matmul,
    and the leftover +sq_row/2 per-partition shift commutes with max8, so
    it is applied to the 8-wide max8 output on the DVE.
"""
import contextlib
import sys

import numpy as np

for _p in ("/opt/trn_rl_repo", "/root/.axon_site/_ro/trn_rl_repo"):
    if _p not in sys.path:
        sys.path.append(_p)

B = 8
N_TOK = 4096
D = 256
P = 128
CHUNK = 512
N_RT = N_TOK // P          # 32 row tiles of 128 tokens
N_CH = N_TOK // CHUNK      # 8 column chunks of 512 tokens
RT_PER_CH = CHUNK // P     # 4 row tiles per chunk
WARMUP_MMS = 128
SAMPLE_K = 256
EPS = 1e-12

_NC = None


def _build():
    import concourse.tile as tile
    from concourse import bacc, masks, mybir

    FP32 = mybir.dt.float32
    FP16 = mybir.dt.float16
    AF = mybir.ActivationFunctionType
    ALU = mybir.AluOpType

    nc = bacc.Bacc("TRN2", target_bir_lowering=False, debug=False, num_devices=B)
    x = nc.dram_tensor("x", [N_TOK, D], FP32, kind="ExternalInput").ap()
    dk2_out = nc.dram_tensor("dk2", [P, N_RT], FP32, kind="ExternalOutput").ap()

    dma_engines = [nc.sync, nc.gpsimd, nc.scalar]

    with tile.TileContext(nc) as tc:
        with contextlib.ExitStack() as ctx:
            const = ctx.enter_context(tc.tile_pool(name="const", bufs=1))
            cast = ctx.enter_context(tc.tile_pool(name="cast", bufs=6))
            sqtmp = ctx.enter_context(tc.tile_pool(name="sqtmp", bufs=3))
            work = ctx.enter_context(tc.tile_pool(name="work", bufs=2))
            candp = ctx.enter_context(tc.tile_pool(name="candp", bufs=1))
            saved = ctx.enter_context(tc.tile_pool(name="saved", bufs=30))
            sq3s = ctx.enter_context(tc.tile_pool(name="sq3s", bufs=3))
            tp = ctx.enter_context(tc.tile_pool(name="tp", bufs=2, space="PSUM"))
            mm = ctx.enter_context(tc.tile_pool(name="mm", bufs=6, space="PSUM"))
            dram = ctx.enter_context(tc.tile_pool(name="dram", bufs=2,
                                                  space="DRAM"))

            x_all = const.tile([P, N_RT * D], FP32)
            xt_h0 = const.tile([P, N_TOK], FP16)
            xt_h1 = const.tile([P, N_TOK], FP16)
            xt_l0 = const.tile([P, N_TOK], FP16)
            xt_l1 = const.tile([P, N_TOK], FP16)
            sq_all = const.tile([P, N_RT], FP32)
            hsq = const.tile([P, N_RT], FP32)       # +sq/2
            sq3row = const.tile([3, N_TOK], FP16)
            ones3 = const.tile([3, P], FP16)
            ident = const.tile([P, P], FP16)
            identf = const.tile([P, P], FP32)
            dk2_all = const.tile([P, N_RT], FP32)
            warm_sink = const.tile([P, P], FP16)

            masks.make_identity(nc, ident[:])
            masks.make_identity(nc, identf[:])
            nc.vector.memset(ones3[:], 1.0)

            # stream the full input in 8 large DMAs (4 row-tiles each)
            for i in range(8):
                seg = x[i * 512:(i + 1) * 512, :].rearrange(
                    "(t p) d -> p t d", t=RT_PER_CH, p=P)
                dst = x_all[:, i * 4 * D:(i + 1) * 4 * D].rearrange(
                    "p (t d) -> p t d", t=RT_PER_CH, d=D)
                dma_engines[i % len(dma_engines)].dma_start(dst, seg)

            def tp_tile(name):
                # one PSUM bank: fp32 [128, 512] with an fp16 [128, 1024] view
                return tp.tile([P, CHUNK], FP32, tag="tpp", name=name)

            # PE warmup: dense real matmuls flip the HAM clock gate to 8/8
            # (transpose-mode does not count as PE-busy for the HAM)
            wt = [tp_tile(f"warm{i}") for i in range(2)]
            for i in range(WARMUP_MMS):
                nc.tensor.matmul(wt[i % 2][:, 0:P], ident[:], ident[:],
                                 start=True, stop=True)
            nc.vector.tensor_copy(warm_sink[:].bitcast(FP32)[:, 0:P // 2],
                                  wt[1][:, 0:P // 2])

            # ---- prepass, per chunk of 4 row tiles ----
            for cc in range(N_CH):
                for tt in range(RT_PER_CH):
                    t = cc * RT_PER_CH + tt
                    xrow = x_all[:, t * D:(t + 1) * D]
                    sqf = sqtmp.tile([P, D], FP32, tag="sqf", name=f"sqf_{t}")
                    nc.scalar.activation(sqf[:], xrow, AF.Square,
                                         accum_out=sq_all[:, t:t + 1])
                    h_t = cast.tile([P, D], FP16, tag="h", name=f"h_{t}")
                    nc.vector.tensor_copy(h_t[:], xrow)
                    l_t = cast.tile([P, D], FP16, tag="l", name=f"l_{t}")
                    nc.vector.scalar_tensor_tensor(l_t[:], xrow, 1.0, h_t[:],
                                                   ALU.mult, ALU.subtract)
                    for src, dst in ((h_t[:, 0:P], xt_h0), (h_t[:, P:D], xt_h1),
                                     (l_t[:, 0:P], xt_l0), (l_t[:, P:D], xt_l1)):
                        tpp = tp_tile(f"tpp_{t}")
                        d16 = tpp[:].bitcast(FP16)[:, 0:P]
                        nc.tensor.transpose(d16, src, ident[:])
                        nc.vector.tensor_copy(dst[:, t * P:(t + 1) * P], d16)

                # 3-level fp16 split of -sq/2 for this chunk's columns,
                # rearranged to [3, 512] rows via a contiguous DRAM bounce
                sl = sq_all[:, cc * RT_PER_CH:(cc + 1) * RT_PER_CH]
                s3c = sq3s.tile([P, 3 * RT_PER_CH], FP16, tag="s3c",
                                name=f"s3c_{cc}")
                s1 = s3c[:, 0 * RT_PER_CH:1 * RT_PER_CH]
                s2 = s3c[:, 1 * RT_PER_CH:2 * RT_PER_CH]
                s3 = s3c[:, 2 * RT_PER_CH:3 * RT_PER_CH]
                r1 = sq3s.tile([P, RT_PER_CH], FP32, tag="r1", name=f"r1_{cc}")
                r2 = sq3s.tile([P, RT_PER_CH], FP32, tag="r2", name=f"r2_{cc}")
                nc.vector.tensor_scalar_mul(s1, sl, -0.5)
                nc.vector.scalar_tensor_tensor(r1[:], sl, -0.5, s1,
                                               ALU.mult, ALU.subtract)
                nc.vector.tensor_copy(s2, r1[:])
                nc.vector.tensor_sub(r2[:], r1[:], s2)
                nc.vector.tensor_copy(s3, r2[:])
                nc.vector.tensor_scalar_mul(
                    hsq[:, cc * RT_PER_CH:(cc + 1) * RT_PER_CH], sl, 0.5)
                tps = tp_tile(f"tps_{cc}")
                t16 = tps[:].bitcast(FP16)[:, 0:P]
                nc.tensor.transpose(t16[:3 * RT_PER_CH, :], s3c[:], ident[:])
                t32 = sq3s.tile([3 * RT_PER_CH, P], FP16, tag="t32",
                                name=f"t32_{cc}")
                nc.vector.tensor_copy(t32[:], t16[:3 * RT_PER_CH, :])
                scr = dram.tile([3 * RT_PER_CH, P], FP16, tag="scr",
                                name=f"scr_{cc}")
                nc.sync.dma_start(scr[:], t32[:])
                for lv in range(3):
                    dst = sq3row[lv:lv + 1,
                                 cc * CHUNK:(cc + 1) * CHUNK].rearrange(
                        "a (m p) -> a m p", m=RT_PER_CH, p=P)
                    src = scr[lv * RT_PER_CH:(lv + 1) * RT_PER_CH,
                              :].unsqueeze(0)
                    nc.sync.dma_start(dst, src)

            # ---- main loop ----
            cands = {}
            saved_tiles = {}

            def rsl(xt, c):
                return xt[:, c * CHUNK:(c + 1) * CHUNK]

            def get_cand(r):
                if r not in cands:
                    cands[r] = candp.tile([P, 64], FP32, tag=f"cand{r}",
                                          name=f"cand_{r}")
                return cands[r]

            for m in range(N_RT):
                c0 = m // 4
                cs_all = list(range(c0, N_CH))
                wh0 = xt_h0[:, m * P:(m + 1) * P]
                wh1 = xt_h1[:, m * P:(m + 1) * P]
                wl0 = xt_l0[:, m * P:(m + 1) * P]
                wl1 = xt_l1[:, m * P:(m + 1) * P]
                cand = get_cand(m)
                for hs in range(0, len(cs_all), 4):
                    cs = cs_all[hs:hs + 4]
                    ps = {}
                    for c in cs:
                        ps[c] = mm.tile([P, CHUNK], FP32, tag="mmps",
                                        name=f"ps_{m}_{c}")
                    for c in cs:
                        nc.tensor.matmul(ps[c][:], wh0, rsl(xt_h0, c),
                                         start=True, stop=False)
                    for c in cs:
                        nc.tensor.matmul(ps[c][:], wh0, rsl(xt_l0, c),
                                         start=False, stop=False)
                    for c in cs:
                        nc.tensor.matmul(ps[c][:], wh1, rsl(xt_h1, c),
                                         start=False, stop=False)
                    for c in cs:
                        nc.tensor.matmul(ps[c][:], wh1, rsl(xt_l1, c),
                                         start=False, stop=False)
                    for c in cs:
                        nc.tensor.matmul(ps[c][:], wl0, rsl(xt_h0, c),
                                         start=False, stop=False)
                    for c in cs:
                        nc.tensor.matmul(ps[c][:], wl1, rsl(xt_h1, c),
                                         start=False, stop=False)
                    for c in cs:
                        nc.tensor.matmul(ps[c][:], ones3[:], rsl(sq3row, c),
                                         start=False, stop=True)
                    for c in cs:
                        nc.vector.max(cand[:, 8 * (c - c0):8 * (c - c0) + 8],
                                      ps[c][:])
                    for c in cs:
                        if c > c0:
                            sv = saved.tile([P, CHUNK], FP32, tag="sv",
                                            name=f"sv_{m}_{c}")
                            nc.scalar.copy(sv[:], ps[c][:])
                            saved_tiles[(m, c)] = sv

                # mirrors sourced from column-block group g = m//4
                if m % 4 == 3:
                    g = m // 4
                    js = [4 * g, 4 * g + 1, 4 * g + 2, 4 * g + 3]
                    for c in range(g + 1, N_CH):
                        for ri in range(RT_PER_CH):
                            r = 4 * c + ri
                            mp = tp_tile(f"mp_{r}_{g}")
                            nc.tensor.matmul(
                                mp[:], ones3[:],
                                sq3row[:, 4 * g * P:(4 * g + 4) * P],
                                start=True, stop=False, skip_group_check=True)
                            for ji, j in enumerate(js):
                                sv = saved_tiles[(j, c)]
                                nc.tensor.matmul(
                                    mp[:, ji * P:(ji + 1) * P],
                                    sv[:, ri * P:(ri + 1) * P], identf[:],
                                    is_transpose=True, start=False,
                                    stop=(ji == 3), skip_group_check=True)
                            # +sq_r/2 commutes with max8: add it to the
                            # 8-wide output instead of the 512-wide tile
                            slot = (N_CH - c) + g
                            rc = get_cand(r)
                            m8 = work.tile([P, 8], FP32, tag="m8",
                                           name=f"m8_{r}_{g}")
                            nc.vector.max(m8[:], mp[:])
                            nc.vector.tensor_scalar_add(
                                rc[:, 8 * slot:8 * slot + 8], m8[:],
                                hsq[:, r:r + 1])

                negs8 = work.tile([P, 8], FP32, tag="negs8", name=f"negs8_{m}")
                nc.vector.max(negs8[:], cand[:])
                nc.vector.scalar_tensor_tensor(dk2_all[:, m:m + 1],
                                               negs8[:, 5:6], -2.0,
                                               sq_all[:, m:m + 1],
                                               ALU.mult, ALU.add)

            nc.sync.dma_start(dk2_out, dk2_all[:])
    nc.compile()
    return nc


def _dk2_on_device(x_np):
    from concourse.bass_utils import run_bass_kernel_spmd
    global _NC
    if _NC is None:
        _NC = _build()
    in_maps = [{"x": np.ascontiguousarray(x_np[b])} for b in range(B)]
    res = run_bass_kernel_spmd(_NC, in_maps, core_ids=list(range(B)))
    # dk2[p, m] holds token 128*m + p
    return np.stack([res.results[b]["dk2"].T.reshape(-1) for b in range(B)])


def kernel(x):
    import jax
    import jax.numpy as jnp

    x_np = np.ascontiguousarray(np.asarray(x, dtype=np.float32))
    dk2 = _dk2_on_device(x_np)

    # host tail on the default jax device — same placement as the reference,
    # so the Gumbel PRNG bits and top_k semantics match bit-for-bit
    w = jnp.sqrt(jnp.asarray(np.maximum(dk2, 0.0), dtype=jnp.float32) + EPS)
    g = jax.random.gumbel(jax.random.key(42), w.shape, dtype=w.dtype)
    scores = jnp.log(w + EPS) + g
    _, idx = jax.lax.top_k(scores, SAMPLE_K)
    idx = np.asarray(idx)
    sampled = np.take_along_axis(x_np, idx[:, :, None], axis=1)
    return (sampled, 0.0)


# revision 3
# speedup vs baseline: 1.0050x; 1.0050x over previous
"""EntropySampler Trainium2 kernel.

Data-parallel over batch: each of the 8 NeuronCores takes one batch item
x_b [4096, 256] and computes dk2[i] = squared distance from token i to its
5th nearest neighbor (excluding self) fully on-device; the cheap O(B*N)
sampling tail (Gumbel noise with key 42, top-256, gather) runs on the host
via jax on the default device, mirroring the reference pipeline's placement
so the PRNG bits match whichever backend the grader uses.

Device algorithm (per core):
  - neg_s[i,j] = dot(x_i,x_j) - sq_j/2 orders each row identically to
    -||x_i - x_j||^2; the diagonal is always rank-0 (self-distance 0), so
    the DVE max8 instruction's 6th entry is the 5th-NN and
    dk2[i] = sq_i - 2*max8[...,5]. No masking, no sort.
  - matmuls run as fp16 hi/lo 3-term products (hh + hl + lh), which
    reproduce the fp32 Gram matrix to ~1e-6 relative error at full
    1-cycle/row PE streaming rate (native fp32 matmul is 4x slower).
  - the per-column -sq_j/2 bias rides a K=3 rank-3 matmul (all-ones lhsT
    against a 3-level fp16 split of -sq/2) accumulated in the same PSUM.
  - symmetry: only chunk-tiles (m, c) with c >= m//4 are computed (144 of
    256). Off-diagonal tiles are copied to SBUF and later PE-transposed in
    128-col slices to serve as below-diagonal candidates for rows 4c..4c+3;
    the column bias of the mirrored view is fixed by another rank-3 matmul,
    and the leftover +sq_row/2 per-partition shift commutes with max8, so
    it is applied to the 8-wide max8 output on the DVE.
"""
import contextlib
import sys

import numpy as np

for _p in ("/opt/trn_rl_repo", "/root/.axon_site/_ro/trn_rl_repo"):
    if _p not in sys.path:
        sys.path.append(_p)

B = 8
N_TOK = 4096
D = 256
P = 128
CHUNK = 512
N_RT = N_TOK // P          # 32 row tiles of 128 tokens
N_CH = N_TOK // CHUNK      # 8 column chunks of 512 tokens
RT_PER_CH = CHUNK // P     # 4 row tiles per chunk
WARMUP_MMS = 40
SAMPLE_K = 256
EPS = 1e-12

_NC = None


def _build():
    import concourse.tile as tile
    from concourse import bacc, masks, mybir

    FP32 = mybir.dt.float32
    FP16 = mybir.dt.float16
    AF = mybir.ActivationFunctionType
    ALU = mybir.AluOpType

    nc = bacc.Bacc("TRN2", target_bir_lowering=False, debug=False, num_devices=B)
    x = nc.dram_tensor("x", [N_TOK, D], FP32, kind="ExternalInput").ap()
    dk2_out = nc.dram_tensor("dk2", [P, N_RT], FP32, kind="ExternalOutput").ap()

    dma_engines = [nc.sync, nc.gpsimd, nc.scalar]

    with tile.TileContext(nc) as tc:
        with contextlib.ExitStack() as ctx:
            const = ctx.enter_context(tc.tile_pool(name="const", bufs=1))
            cast = ctx.enter_context(tc.tile_pool(name="cast", bufs=6))
            sqtmp = ctx.enter_context(tc.tile_pool(name="sqtmp", bufs=3))
            work = ctx.enter_context(tc.tile_pool(name="work", bufs=2))
            candp = ctx.enter_context(tc.tile_pool(name="candp", bufs=1))
            saved = ctx.enter_context(tc.tile_pool(name="saved", bufs=30))
            sq3s = ctx.enter_context(tc.tile_pool(name="sq3s", bufs=3))
            tp = ctx.enter_context(tc.tile_pool(name="tp", bufs=2, space="PSUM"))
            mm = ctx.enter_context(tc.tile_pool(name="mm", bufs=6, space="PSUM"))
            dram = ctx.enter_context(tc.tile_pool(name="dram", bufs=2,
                                                  space="DRAM"))

            x_all = const.tile([P, N_RT * D], FP32)
            xt_h0 = const.tile([P, N_TOK], FP16)
            xt_h1 = const.tile([P, N_TOK], FP16)
            xt_l0 = const.tile([P, N_TOK], FP16)
            xt_l1 = const.tile([P, N_TOK], FP16)
            sq_all = const.tile([P, N_RT], FP32)
            hsq = const.tile([P, N_RT], FP32)       # +sq/2
            sq3row = const.tile([3, N_TOK], FP16)
            ones3 = const.tile([3, P], FP16)
            ident = const.tile([P, P], FP16)
            identf = const.tile([P, P], FP32)
            dk2_all = const.tile([P, N_RT], FP32)
            warm_sink = const.tile([P, P], FP16)

            masks.make_identity(nc, ident[:])
            masks.make_identity(nc, identf[:])
            nc.vector.memset(ones3[:], 1.0)

            # stream the full input in 8 large DMAs (4 row-tiles each)
            for i in range(8):
                seg = x[i * 512:(i + 1) * 512, :].rearrange(
                    "(t p) d -> p t d", t=RT_PER_CH, p=P)
                dst = x_all[:, i * 4 * D:(i + 1) * 4 * D].rearrange(
                    "p (t d) -> p t d", t=RT_PER_CH, d=D)
                dma_engines[i % len(dma_engines)].dma_start(dst, seg)

            def tp_tile(name):
                # one PSUM bank: fp32 [128, 512] with an fp16 [128, 1024] view
                return tp.tile([P, CHUNK], FP32, tag="tpp", name=name)

            # PE warmup: dense real matmuls flip the HAM clock gate to 8/8
            # (transpose-mode does not count as PE-busy for the HAM). Uses an
            # mm-pool slot so the tp pool stays free for prepass transposes.
            wt = mm.tile([P, CHUNK], FP32, tag="mmps", name="warm0")
            for i in range(WARMUP_MMS):
                nc.tensor.matmul(wt[:, 0:P], ident[:], ident[:],
                                 start=True, stop=True)
            nc.vector.tensor_copy(warm_sink[:].bitcast(FP32)[:, 0:P // 2],
                                  wt[:, 0:P // 2])

            # ---- prepass, per chunk of 4 row tiles ----
            for cc in range(N_CH):
                for tt in range(RT_PER_CH):
                    t = cc * RT_PER_CH + tt
                    xrow = x_all[:, t * D:(t + 1) * D]
                    sqf = sqtmp.tile([P, D], FP32, tag="sqf", name=f"sqf_{t}")
                    nc.scalar.activation(sqf[:], xrow, AF.Square,
                                         accum_out=sq_all[:, t:t + 1])
                    h_t = cast.tile([P, D], FP16, tag="h", name=f"h_{t}")
                    nc.vector.tensor_copy(h_t[:], xrow)
                    l_t = cast.tile([P, D], FP16, tag="l", name=f"l_{t}")
                    nc.vector.scalar_tensor_tensor(l_t[:], xrow, 1.0, h_t[:],
                                                   ALU.mult, ALU.subtract)
                    for src, dst in ((h_t[:, 0:P], xt_h0), (h_t[:, P:D], xt_h1),
                                     (l_t[:, 0:P], xt_l0), (l_t[:, P:D], xt_l1)):
                        tpp = tp_tile(f"tpp_{t}")
                        d16 = tpp[:].bitcast(FP16)[:, 0:P]
                        nc.tensor.transpose(d16, src, ident[:])
                        nc.vector.tensor_copy(dst[:, t * P:(t + 1) * P], d16)

                # 3-level fp16 split of -sq/2 for this chunk's columns,
                # rearranged to [3, 512] rows via a contiguous DRAM bounce
                sl = sq_all[:, cc * RT_PER_CH:(cc + 1) * RT_PER_CH]
                s3c = sq3s.tile([P, 3 * RT_PER_CH], FP16, tag="s3c",
                                name=f"s3c_{cc}")
                s1 = s3c[:, 0 * RT_PER_CH:1 * RT_PER_CH]
                s2 = s3c[:, 1 * RT_PER_CH:2 * RT_PER_CH]
                s3 = s3c[:, 2 * RT_PER_CH:3 * RT_PER_CH]
                r1 = sq3s.tile([P, RT_PER_CH], FP32, tag="r1", name=f"r1_{cc}")
                r2 = sq3s.tile([P, RT_PER_CH], FP32, tag="r2", name=f"r2_{cc}")
                nc.vector.tensor_scalar_mul(s1, sl, -0.5)
                nc.vector.scalar_tensor_tensor(r1[:], sl, -0.5, s1,
                                               ALU.mult, ALU.subtract)
                nc.vector.tensor_copy(s2, r1[:])
                nc.vector.tensor_sub(r2[:], r1[:], s2)
                nc.vector.tensor_copy(s3, r2[:])
                nc.vector.tensor_scalar_mul(
                    hsq[:, cc * RT_PER_CH:(cc + 1) * RT_PER_CH], sl, 0.5)
                tps = tp_tile(f"tps_{cc}")
                t16 = tps[:].bitcast(FP16)[:, 0:P]
                nc.tensor.transpose(t16[:3 * RT_PER_CH, :], s3c[:], ident[:])
                t32 = sq3s.tile([3 * RT_PER_CH, P], FP16, tag="t32",
                                name=f"t32_{cc}")
                nc.vector.tensor_copy(t32[:], t16[:3 * RT_PER_CH, :])
                scr = dram.tile([3 * RT_PER_CH, P], FP16, tag="scr",
                                name=f"scr_{cc}")
                nc.sync.dma_start(scr[:], t32[:])
                for lv in range(3):
                    dst = sq3row[lv:lv + 1,
                                 cc * CHUNK:(cc + 1) * CHUNK].rearrange(
                        "a (m p) -> a m p", m=RT_PER_CH, p=P)
                    src = scr[lv * RT_PER_CH:(lv + 1) * RT_PER_CH,
                              :].unsqueeze(0)
                    nc.sync.dma_start(dst, src)

            # ---- main loop ----
            cands = {}
            saved_tiles = {}

            def rsl(xt, c):
                return xt[:, c * CHUNK:(c + 1) * CHUNK]

            def get_cand(r):
                if r not in cands:
                    cands[r] = candp.tile([P, 64], FP32, tag=f"cand{r}",
                                          name=f"cand_{r}")
                return cands[r]

            for m in range(N_RT):
                c0 = m // 4
                cs_all = list(range(c0, N_CH))
                wh0 = xt_h0[:, m * P:(m + 1) * P]
                wh1 = xt_h1[:, m * P:(m + 1) * P]
                wl0 = xt_l0[:, m * P:(m + 1) * P]
                wl1 = xt_l1[:, m * P:(m + 1) * P]
                cand = get_cand(m)
                for hs in range(0, len(cs_all), 4):
                    cs = cs_all[hs:hs + 4]
                    ps = {}
                    for c in cs:
                        ps[c] = mm.tile([P, CHUNK], FP32, tag="mmps",
                                        name=f"ps_{m}_{c}")
                    for c in cs:
                        nc.tensor.matmul(ps[c][:], wh0, rsl(xt_h0, c),
                                         start=True, stop=False)
                    for c in cs:
                        nc.tensor.matmul(ps[c][:], wh0, rsl(xt_l0, c),
                                         start=False, stop=False)
                    for c in cs:
                        nc.tensor.matmul(ps[c][:], wh1, rsl(xt_h1, c),
                                         start=False, stop=False)
                    for c in cs:
                        nc.tensor.matmul(ps[c][:], wh1, rsl(xt_l1, c),
                                         start=False, stop=False)
                    for c in cs:
                        nc.tensor.matmul(ps[c][:], wl0, rsl(xt_h0, c),
                                         start=False, stop=False)
                    for c in cs:
                        nc.tensor.matmul(ps[c][:], wl1, rsl(xt_h1, c),
                                         start=False, stop=False)
                    for c in cs:
                        nc.tensor.matmul(ps[c][:], ones3[:], rsl(sq3row, c),
                                         start=False, stop=True)
                    for c in cs:
                        nc.vector.max(cand[:, 8 * (c - c0):8 * (c - c0) + 8],
                                      ps[c][:])
                    for c in cs:
                        if c > c0:
                            sv = saved.tile([P, CHUNK], FP32, tag="sv",
                                            name=f"sv_{m}_{c}")
                            nc.scalar.copy(sv[:], ps[c][:])
                            saved_tiles[(m, c)] = sv

                # mirrors sourced from column-block group g = m//4
                if m % 4 == 3:
                    g = m // 4
                    js = [4 * g, 4 * g + 1, 4 * g + 2, 4 * g + 3]
                    for c in range(g + 1, N_CH):
                        for ri in range(RT_PER_CH):
                            r = 4 * c + ri
                            mp = tp_tile(f"mp_{r}_{g}")
                            nc.tensor.matmul(
                                mp[:], ones3[:],
                                sq3row[:, 4 * g * P:(4 * g + 4) * P],
                                start=True, stop=False, skip_group_check=True)
                            for ji, j in enumerate(js):
                                sv = saved_tiles[(j, c)]
                                nc.tensor.matmul(
                                    mp[:, ji * P:(ji + 1) * P],
                                    sv[:, ri * P:(ri + 1) * P], identf[:],
                                    is_transpose=True, start=False,
                                    stop=(ji == 3), skip_group_check=True)
                            # +sq_r/2 commutes with max8: add it to the
                            # 8-wide output instead of the 512-wide tile
                            slot = (N_CH - c) + g
                            rc = get_cand(r)
                            m8 = work.tile([P, 8], FP32, tag="m8",
                                           name=f"m8_{r}_{g}")
                            nc.vector.max(m8[:], mp[:])
                            nc.vector.tensor_scalar_add(
                                rc[:, 8 * slot:8 * slot + 8], m8[:],
                                hsq[:, r:r + 1])

                negs8 = work.tile([P, 8], FP32, tag="negs8", name=f"negs8_{m}")
                nc.vector.max(negs8[:], cand[:])
                nc.vector.scalar_tensor_tensor(dk2_all[:, m:m + 1],
                                               negs8[:, 5:6], -2.0,
                                               sq_all[:, m:m + 1],
                                               ALU.mult, ALU.add)

            nc.sync.dma_start(dk2_out, dk2_all[:])
    nc.compile()
    return nc


def _dk2_on_device(x_np):
    from concourse.bass_utils import run_bass_kernel_spmd
    global _NC
    if _NC is None:
        _NC = _build()
    in_maps = [{"x": np.ascontiguousarray(x_np[b])} for b in range(B)]
    res = run_bass_kernel_spmd(_NC, in_maps, core_ids=list(range(B)))
    # dk2[p, m] holds token 128*m + p
    return np.stack([res.results[b]["dk2"].T.reshape(-1) for b in range(B)])


def kernel(x):
    import jax
    import jax.numpy as jnp

    x_np = np.ascontiguousarray(np.asarray(x, dtype=np.float32))
    dk2 = _dk2_on_device(x_np)

    # host tail on the default jax device — same placement as the reference,
    # so the Gumbel PRNG bits and top_k semantics match bit-for-bit
    w = jnp.sqrt(jnp.asarray(np.maximum(dk2, 0.0), dtype=jnp.float32) + EPS)
    g = jax.random.gumbel(jax.random.key(42), w.shape, dtype=w.dtype)
    scores = jnp.log(w + EPS) + g
    _, idx = jax.lax.top_k(scores, SAMPLE_K)
    idx = np.asarray(idx)
    sampled = np.take_along_axis(x_np, idx[:, :, None], axis=1)
    return (sampled, 0.0)


# revision 4
# speedup vs baseline: 1.0141x; 1.0091x over previous
"""EntropySampler Trainium2 kernel.

Data-parallel over batch: each of the 8 NeuronCores takes one batch item
x_b [4096, 256] and computes dk2[i] = squared distance from token i to its
5th nearest neighbor (excluding self) fully on-device; the cheap O(B*N)
sampling tail (Gumbel noise with key 42, top-256, gather) runs on the host
via jax on the default device, mirroring the reference pipeline's placement
so the PRNG bits match whichever backend the grader uses.

Device algorithm (per core):
  - neg_s[i,j] = dot(x_i,x_j) - sq_j/2 orders each row identically to
    -||x_i - x_j||^2; the diagonal is always rank-0 (self-distance 0), so
    the DVE max8 instruction's 6th entry is the 5th-NN and
    dk2[i] = sq_i - 2*max8[...,5]. No masking, no sort.
  - matmuls run as fp16 hi/lo 3-term products (hh + hl + lh), which
    reproduce the fp32 Gram matrix to ~1e-6 relative error at full
    1-cycle/row PE streaming rate (native fp32 matmul is 4x slower).
  - the per-column -sq_j/2 bias rides a K=3 rank-3 matmul (all-ones lhsT
    against a 3-level fp16 split of -sq/2) accumulated in the same PSUM.
  - symmetry: only chunk-tiles (m, c) with c >= m//4 are computed (144 of
    256). Off-diagonal tiles are copied to SBUF and later PE-transposed in
    128-col slices to serve as below-diagonal candidates for rows 4c..4c+3;
    the column bias of the mirrored view is fixed by another rank-3 matmul,
    and the leftover +sq_row/2 per-partition shift commutes with max8, so
    it is applied to the 8-wide max8 output on the DVE.
"""
import contextlib
import sys

import numpy as np

for _p in ("/opt/trn_rl_repo", "/root/.axon_site/_ro/trn_rl_repo"):
    if _p not in sys.path:
        sys.path.append(_p)

B = 8
N_TOK = 4096
D = 256
P = 128
CHUNK = 512
N_RT = N_TOK // P          # 32 row tiles of 128 tokens
N_CH = N_TOK // CHUNK      # 8 column chunks of 512 tokens
RT_PER_CH = CHUNK // P     # 4 row tiles per chunk
WARMUP_MMS = 40
SAMPLE_K = 256
EPS = 1e-12

_NC = None


def _build():
    import concourse.tile as tile
    from concourse import bacc, masks, mybir

    FP32 = mybir.dt.float32
    FP16 = mybir.dt.float16
    AF = mybir.ActivationFunctionType
    ALU = mybir.AluOpType

    nc = bacc.Bacc("TRN2", target_bir_lowering=False, debug=False, num_devices=B)
    x = nc.dram_tensor("x", [N_TOK, D], FP32, kind="ExternalInput").ap()
    dk2_out = nc.dram_tensor("dk2", [P, N_RT], FP32, kind="ExternalOutput").ap()

    dma_engines = [nc.sync, nc.gpsimd, nc.scalar]

    with tile.TileContext(nc) as tc:
        with contextlib.ExitStack() as ctx:
            const = ctx.enter_context(tc.tile_pool(name="const", bufs=1))
            cast = ctx.enter_context(tc.tile_pool(name="cast", bufs=6))
            sqtmp = ctx.enter_context(tc.tile_pool(name="sqtmp", bufs=3))
            work = ctx.enter_context(tc.tile_pool(name="work", bufs=2))
            candp = ctx.enter_context(tc.tile_pool(name="candp", bufs=1))
            saved = ctx.enter_context(tc.tile_pool(name="saved", bufs=30))
            sq3s = ctx.enter_context(tc.tile_pool(name="sq3s", bufs=3))
            tp = ctx.enter_context(tc.tile_pool(name="tp", bufs=4, space="PSUM"))
            mm = ctx.enter_context(tc.tile_pool(name="mm", bufs=4, space="PSUM"))
            dram = ctx.enter_context(tc.tile_pool(name="dram", bufs=2,
                                                  space="DRAM"))

            x_all = const.tile([P, N_RT * D], FP32)
            xt_h0 = const.tile([P, N_TOK], FP16)
            xt_h1 = const.tile([P, N_TOK], FP16)
            xt_l0 = const.tile([P, N_TOK], FP16)
            xt_l1 = const.tile([P, N_TOK], FP16)
            sq_all = const.tile([P, N_RT], FP32)
            hsq = const.tile([P, N_RT], FP32)       # +sq/2
            sq3row = const.tile([3, N_TOK], FP16)
            ones3 = const.tile([3, P], FP16)
            ident = const.tile([P, P], FP16)
            identf = const.tile([P, P], FP32)
            dk2_all = const.tile([P, N_RT], FP32)
            warm_sink = const.tile([P, P], FP16)

            masks.make_identity(nc, ident[:])
            masks.make_identity(nc, identf[:])
            nc.vector.memset(ones3[:], 1.0)

            # stream the full input in 8 large DMAs (4 row-tiles each)
            for i in range(8):
                seg = x[i * 512:(i + 1) * 512, :].rearrange(
                    "(t p) d -> p t d", t=RT_PER_CH, p=P)
                dst = x_all[:, i * 4 * D:(i + 1) * 4 * D].rearrange(
                    "p (t d) -> p t d", t=RT_PER_CH, d=D)
                dma_engines[i % len(dma_engines)].dma_start(dst, seg)

            def tp_tile(name):
                # one PSUM bank: fp32 [128, 512] with an fp16 [128, 1024] view
                return tp.tile([P, CHUNK], FP32, tag="tpp", name=name)

            # PE warmup: dense real matmuls flip the HAM clock gate to 8/8
            # (transpose-mode does not count as PE-busy for the HAM). Uses an
            # mm-pool slot so the tp pool stays free for prepass transposes.
            wt = mm.tile([P, CHUNK], FP32, tag="mmps", name="warm0")
            for i in range(WARMUP_MMS):
                nc.tensor.matmul(wt[:, 0:P], ident[:], ident[:],
                                 start=True, stop=True)
            nc.vector.tensor_copy(warm_sink[:].bitcast(FP32)[:, 0:P // 2],
                                  wt[:, 0:P // 2])

            # ---- prepass, per chunk of 4 row tiles ----
            for cc in range(N_CH):
                for tt in range(RT_PER_CH):
                    t = cc * RT_PER_CH + tt
                    xrow = x_all[:, t * D:(t + 1) * D]
                    sqf = sqtmp.tile([P, D], FP32, tag="sqf", name=f"sqf_{t}")
                    nc.scalar.activation(sqf[:], xrow, AF.Square,
                                         accum_out=sq_all[:, t:t + 1])
                    h_t = cast.tile([P, D], FP16, tag="h", name=f"h_{t}")
                    nc.vector.tensor_copy(h_t[:], xrow)
                    l_t = cast.tile([P, D], FP16, tag="l", name=f"l_{t}")
                    nc.vector.scalar_tensor_tensor(l_t[:], xrow, 1.0, h_t[:],
                                                   ALU.mult, ALU.subtract)
                    for src, dst in ((h_t[:, 0:P], xt_h0), (h_t[:, P:D], xt_h1),
                                     (l_t[:, 0:P], xt_l0), (l_t[:, P:D], xt_l1)):
                        tpp = tp_tile(f"tpp_{t}")
                        d16 = tpp[:].bitcast(FP16)[:, 0:P]
                        nc.tensor.transpose(d16, src, ident[:])
                        nc.vector.tensor_copy(dst[:, t * P:(t + 1) * P], d16)

                # 3-level fp16 split of -sq/2 for this chunk's columns,
                # rearranged to [3, 512] rows via a contiguous DRAM bounce
                sl = sq_all[:, cc * RT_PER_CH:(cc + 1) * RT_PER_CH]
                s3c = sq3s.tile([P, 3 * RT_PER_CH], FP16, tag="s3c",
                                name=f"s3c_{cc}")
                s1 = s3c[:, 0 * RT_PER_CH:1 * RT_PER_CH]
                s2 = s3c[:, 1 * RT_PER_CH:2 * RT_PER_CH]
                s3 = s3c[:, 2 * RT_PER_CH:3 * RT_PER_CH]
                r1 = sq3s.tile([P, RT_PER_CH], FP32, tag="r1", name=f"r1_{cc}")
                r2 = sq3s.tile([P, RT_PER_CH], FP32, tag="r2", name=f"r2_{cc}")
                nc.vector.tensor_scalar_mul(s1, sl, -0.5)
                nc.vector.scalar_tensor_tensor(r1[:], sl, -0.5, s1,
                                               ALU.mult, ALU.subtract)
                nc.vector.tensor_copy(s2, r1[:])
                nc.vector.tensor_sub(r2[:], r1[:], s2)
                nc.vector.tensor_copy(s3, r2[:])
                nc.vector.tensor_scalar_mul(
                    hsq[:, cc * RT_PER_CH:(cc + 1) * RT_PER_CH], sl, 0.5)
                tps = tp_tile(f"tps_{cc}")
                t16 = tps[:].bitcast(FP16)[:, 0:P]
                nc.tensor.transpose(t16[:3 * RT_PER_CH, :], s3c[:], ident[:])
                t32 = sq3s.tile([3 * RT_PER_CH, P], FP16, tag="t32",
                                name=f"t32_{cc}")
                nc.vector.tensor_copy(t32[:], t16[:3 * RT_PER_CH, :])
                scr = dram.tile([3 * RT_PER_CH, P], FP16, tag="scr",
                                name=f"scr_{cc}")
                nc.sync.dma_start(scr[:], t32[:])
                for lv in range(3):
                    dst = sq3row[lv:lv + 1,
                                 cc * CHUNK:(cc + 1) * CHUNK].rearrange(
                        "a (m p) -> a m p", m=RT_PER_CH, p=P)
                    src = scr[lv * RT_PER_CH:(lv + 1) * RT_PER_CH,
                              :].unsqueeze(0)
                    nc.sync.dma_start(dst, src)

            # ---- main loop ----
            cands = {}
            saved_tiles = {}

            def rsl(xt, c):
                return xt[:, c * CHUNK:(c + 1) * CHUNK]

            def get_cand(r):
                if r not in cands:
                    cands[r] = candp.tile([P, 64], FP32, tag=f"cand{r}",
                                          name=f"cand_{r}")
                return cands[r]

            for m in range(N_RT):
                c0 = m // 4
                cs_all = list(range(c0, N_CH))
                wh0 = xt_h0[:, m * P:(m + 1) * P]
                wh1 = xt_h1[:, m * P:(m + 1) * P]
                wl0 = xt_l0[:, m * P:(m + 1) * P]
                wl1 = xt_l1[:, m * P:(m + 1) * P]
                cand = get_cand(m)
                for hs in range(0, len(cs_all), 4):
                    cs = cs_all[hs:hs + 4]
                    ps = {}
                    for c in cs:
                        ps[c] = mm.tile([P, CHUNK], FP32, tag="mmps",
                                        name=f"ps_{m}_{c}")
                    for c in cs:
                        nc.tensor.matmul(ps[c][:], wh0, rsl(xt_h0, c),
                                         start=True, stop=False)
                    for c in cs:
                        nc.tensor.matmul(ps[c][:], wh0, rsl(xt_l0, c),
                                         start=False, stop=False)
                    for c in cs:
                        nc.tensor.matmul(ps[c][:], wh1, rsl(xt_h1, c),
                                         start=False, stop=False)
                    for c in cs:
                        nc.tensor.matmul(ps[c][:], wh1, rsl(xt_l1, c),
                                         start=False, stop=False)
                    for c in cs:
                        nc.tensor.matmul(ps[c][:], wl0, rsl(xt_h0, c),
                                         start=False, stop=False)
                    for c in cs:
                        nc.tensor.matmul(ps[c][:], wl1, rsl(xt_h1, c),
                                         start=False, stop=False)
                    for c in cs:
                        nc.tensor.matmul(ps[c][:], ones3[:], rsl(sq3row, c),
                                         start=False, stop=True)
                    for c in cs:
                        nc.vector.max(cand[:, 8 * (c - c0):8 * (c - c0) + 8],
                                      ps[c][:])
                    for c in cs:
                        if c > c0:
                            sv = saved.tile([P, CHUNK], FP32, tag="sv",
                                            name=f"sv_{m}_{c}")
                            nc.scalar.copy(sv[:], ps[c][:])
                            saved_tiles[(m, c)] = sv

                # mirrors sourced from column-block group g = m//4
                if m % 4 == 3:
                    g = m // 4
                    js = [4 * g, 4 * g + 1, 4 * g + 2, 4 * g + 3]
                    for c in range(g + 1, N_CH):
                        for ri in range(RT_PER_CH):
                            r = 4 * c + ri
                            mp = tp_tile(f"mp_{r}_{g}")
                            nc.tensor.matmul(
                                mp[:], ones3[:],
                                sq3row[:, 4 * g * P:(4 * g + 4) * P],
                                start=True, stop=False, skip_group_check=True)
                            for ji, j in enumerate(js):
                                sv = saved_tiles[(j, c)]
                                nc.tensor.matmul(
                                    mp[:, ji * P:(ji + 1) * P],
                                    sv[:, ri * P:(ri + 1) * P], identf[:],
                                    is_transpose=True, start=False,
                                    stop=(ji == 3), skip_group_check=True)
                            # +sq_r/2 commutes with max8: add it to the
                            # 8-wide output instead of the 512-wide tile
                            slot = (N_CH - c) + g
                            rc = get_cand(r)
                            m8 = work.tile([P, 8], FP32, tag="m8",
                                           name=f"m8_{r}_{g}")
                            nc.vector.max(m8[:], mp[:])
                            nc.vector.tensor_scalar_add(
                                rc[:, 8 * slot:8 * slot + 8], m8[:],
                                hsq[:, r:r + 1])

                negs8 = work.tile([P, 8], FP32, tag="negs8", name=f"negs8_{m}")
                nc.vector.max(negs8[:], cand[:])
                nc.vector.scalar_tensor_tensor(dk2_all[:, m:m + 1],
                                               negs8[:, 5:6], -2.0,
                                               sq_all[:, m:m + 1],
                                               ALU.mult, ALU.add)

            nc.sync.dma_start(dk2_out, dk2_all[:])
    nc.compile()
    return nc


def _dk2_on_device(x_np):
    from concourse.bass_utils import run_bass_kernel_spmd
    global _NC
    if _NC is None:
        _NC = _build()
    in_maps = [{"x": np.ascontiguousarray(x_np[b])} for b in range(B)]
    res = run_bass_kernel_spmd(_NC, in_maps, core_ids=list(range(B)))
    # dk2[p, m] holds token 128*m + p
    return np.stack([res.results[b]["dk2"].T.reshape(-1) for b in range(B)])


def kernel(x):
    import jax
    import jax.numpy as jnp

    x_np = np.ascontiguousarray(np.asarray(x, dtype=np.float32))
    dk2 = _dk2_on_device(x_np)

    # host tail on the default jax device — same placement as the reference,
    # so the Gumbel PRNG bits and top_k semantics match bit-for-bit
    w = jnp.sqrt(jnp.asarray(np.maximum(dk2, 0.0), dtype=jnp.float32) + EPS)
    g = jax.random.gumbel(jax.random.key(42), w.shape, dtype=w.dtype)
    scores = jnp.log(w + EPS) + g
    _, idx = jax.lax.top_k(scores, SAMPLE_K)
    idx = np.asarray(idx)
    sampled = np.take_along_axis(x_np, idx[:, :, None], axis=1)
    return (sampled, 0.0)


# revision 5
# speedup vs baseline: 1.0443x; 1.0298x over previous
"""EntropySampler Trainium2 kernel.

Data-parallel over batch: each of the 8 NeuronCores takes one batch item
x_b [4096, 256] and computes dk2[i] = squared distance from token i to its
5th nearest neighbor (excluding self) fully on-device; the cheap O(B*N)
sampling tail (Gumbel noise with key 42, top-256, gather) runs on the host
via jax on the default device, mirroring the reference pipeline's placement
so the PRNG bits match whichever backend the grader uses.

Device algorithm (per core):
  - neg_s[i,j] = dot(x_i,x_j) - sq_j/2 orders each row identically to
    -||x_i - x_j||^2; the diagonal is always rank-0 (self-distance 0), so
    the DVE max8 instruction's 6th entry is the 5th-NN and
    dk2[i] = sq_i - 2*max8[...,5]. No masking, no sort.
  - matmuls run as fp16 hi/lo 3-term products (hh + hl + lh), which
    reproduce the fp32 Gram matrix to ~1e-6 relative error at full
    1-cycle/row PE streaming rate (native fp32 matmul is 4x slower).
  - the per-column -sq_j/2 bias rides a K=3 rank-3 matmul (all-ones lhsT
    against a 3-level fp16 split of -sq/2) accumulated in the same PSUM.
  - symmetry: only chunk-tiles (m, c) with c >= m//4 are computed (144 of
    256). Off-diagonal tiles are copied to SBUF and later PE-transposed in
    128-col slices to serve as below-diagonal candidates for rows 4c..4c+3;
    the column bias of the mirrored view is fixed by another rank-3 matmul,
    and the leftover +sq_row/2 per-partition shift commutes with max8, so
    it is applied to the 8-wide max8 output on the DVE.
"""
import contextlib
import sys

import numpy as np

for _p in ("/opt/trn_rl_repo", "/root/.axon_site/_ro/trn_rl_repo"):
    if _p not in sys.path:
        sys.path.append(_p)

B = 8
N_TOK = 4096
D = 256
P = 128
CHUNK = 512
N_RT = N_TOK // P          # 32 row tiles of 128 tokens
N_CH = N_TOK // CHUNK      # 8 column chunks of 512 tokens
RT_PER_CH = CHUNK // P     # 4 row tiles per chunk
WARMUP_MMS = 40
SAMPLE_K = 256
EPS = 1e-12

_NC = None


def _build():
    import concourse.tile as tile
    from concourse import bacc, masks, mybir

    FP32 = mybir.dt.float32
    FP16 = mybir.dt.float16
    AF = mybir.ActivationFunctionType
    ALU = mybir.AluOpType

    nc = bacc.Bacc("TRN2", target_bir_lowering=False, debug=False, num_devices=B)
    x = nc.dram_tensor("x", [N_TOK, D], FP32, kind="ExternalInput").ap()
    dk2_out = nc.dram_tensor("dk2", [P, N_RT], FP32, kind="ExternalOutput").ap()

    dma_engines = [nc.sync, nc.gpsimd, nc.scalar]

    with tile.TileContext(nc) as tc:
        with contextlib.ExitStack() as ctx:
            const = ctx.enter_context(tc.tile_pool(name="const", bufs=1))
            cast = ctx.enter_context(tc.tile_pool(name="cast", bufs=5))
            sqtmp = ctx.enter_context(tc.tile_pool(name="sqtmp", bufs=3))
            work = ctx.enter_context(tc.tile_pool(name="work", bufs=2))
            candp = ctx.enter_context(tc.tile_pool(name="candp", bufs=1))
            saved = ctx.enter_context(tc.tile_pool(name="saved", bufs=30))
            sq3s = ctx.enter_context(tc.tile_pool(name="sq3s", bufs=3))
            tp = ctx.enter_context(tc.tile_pool(name="tp", bufs=4, space="PSUM"))
            mm = ctx.enter_context(tc.tile_pool(name="mm", bufs=4, space="PSUM"))
            dram = ctx.enter_context(tc.tile_pool(name="dram", bufs=2,
                                                  space="DRAM"))

            x_all = const.tile([P, N_RT * D], FP32)
            xt_h0 = const.tile([P, N_TOK], FP16)
            xt_h1 = const.tile([P, N_TOK], FP16)
            xt_l0 = const.tile([P, N_TOK], FP16)
            xt_l1 = const.tile([P, N_TOK], FP16)
            sq_all = const.tile([P, N_RT], FP32)
            hsq = const.tile([P, N_RT], FP32)       # +sq/2
            sq3row = const.tile([3, N_TOK], FP16)
            ones3 = const.tile([3, P], FP16)
            ident = const.tile([P, P], FP16)
            identf = const.tile([P, P], FP32)
            dk2_all = const.tile([P, N_RT], FP32)
            warm_sink = const.tile([P, P], FP16)

            masks.make_identity(nc, ident[:])
            masks.make_identity(nc, identf[:])
            nc.vector.memset(ones3[:], 1.0)

            # stream the full input in 8 large DMAs (4 row-tiles each)
            for i in range(8):
                seg = x[i * 512:(i + 1) * 512, :].rearrange(
                    "(t p) d -> p t d", t=RT_PER_CH, p=P)
                dst = x_all[:, i * 4 * D:(i + 1) * 4 * D].rearrange(
                    "p (t d) -> p t d", t=RT_PER_CH, d=D)
                dma_engines[i % len(dma_engines)].dma_start(dst, seg)

            def tp_tile(name):
                # one PSUM bank: fp32 [128, 512] with an fp16 [128, 1024] view
                return tp.tile([P, CHUNK], FP32, tag="tpp", name=name)

            # PE warmup: dense real matmuls flip the HAM clock gate to 8/8
            # (transpose-mode does not count as PE-busy for the HAM). Uses an
            # mm-pool slot so the tp pool stays free for prepass transposes.
            wt = mm.tile([P, CHUNK], FP32, tag="mmps", name="warm0")
            for i in range(WARMUP_MMS):
                nc.tensor.matmul(wt[:, 0:P], ident[:], ident[:],
                                 start=True, stop=True)
            nc.vector.tensor_copy(warm_sink[:].bitcast(FP32)[:, 0:P // 2],
                                  wt[:, 0:P // 2])

            # ---- prepass, per chunk of 4 row tiles ----
            for cc in range(N_CH):
                hs, ls = [], []
                for tt in range(RT_PER_CH):
                    t = cc * RT_PER_CH + tt
                    xrow = x_all[:, t * D:(t + 1) * D]
                    sqf = sqtmp.tile([P, D], FP32, tag="sqf", name=f"sqf_{t}")
                    nc.scalar.activation(sqf[:], xrow, AF.Square,
                                         accum_out=sq_all[:, t:t + 1])
                    h_t = cast.tile([P, D], FP16, tag="h", name=f"h_{t}")
                    nc.vector.tensor_copy(h_t[:], xrow)
                    l_t = cast.tile([P, D], FP16, tag="l", name=f"l_{t}")
                    nc.vector.scalar_tensor_tensor(l_t[:], xrow, 1.0, h_t[:],
                                                   ALU.mult, ALU.subtract)
                    hs.append(h_t)
                    ls.append(l_t)
                # pack the chunk's 4 transposes per tensor-part into one
                # PSUM bank -> a single 512-wide DVE copy per part (the DVE
                # copy chain paced the startup window at 4x [128,128] copies)
                for pi, (arr, lo, dstT) in enumerate(
                        ((hs, 0, xt_h0), (hs, P, xt_h1),
                         (ls, 0, xt_l0), (ls, P, xt_l1))):
                    grp = tp_tile(f"grp_{cc}_{pi}")
                    g16 = grp[:].bitcast(FP16)
                    for tt in range(RT_PER_CH):
                        nc.tensor.transpose(g16[:, tt * P:(tt + 1) * P],
                                            arr[tt][:, lo:lo + P], ident[:])
                    nc.vector.tensor_copy(
                        dstT[:, cc * CHUNK:(cc + 1) * CHUNK], g16[:, 0:CHUNK])

                # 3-level fp16 split of -sq/2 for this chunk's columns,
                # rearranged to [3, 512] rows via a contiguous DRAM bounce
                sl = sq_all[:, cc * RT_PER_CH:(cc + 1) * RT_PER_CH]
                s3c = sq3s.tile([P, 3 * RT_PER_CH], FP16, tag="s3c",
                                name=f"s3c_{cc}")
                s1 = s3c[:, 0 * RT_PER_CH:1 * RT_PER_CH]
                s2 = s3c[:, 1 * RT_PER_CH:2 * RT_PER_CH]
                s3 = s3c[:, 2 * RT_PER_CH:3 * RT_PER_CH]
                r1 = sq3s.tile([P, RT_PER_CH], FP32, tag="r1", name=f"r1_{cc}")
                r2 = sq3s.tile([P, RT_PER_CH], FP32, tag="r2", name=f"r2_{cc}")
                nc.vector.tensor_scalar_mul(s1, sl, -0.5)
                nc.vector.scalar_tensor_tensor(r1[:], sl, -0.5, s1,
                                               ALU.mult, ALU.subtract)
                nc.vector.tensor_copy(s2, r1[:])
                nc.vector.tensor_sub(r2[:], r1[:], s2)
                nc.vector.tensor_copy(s3, r2[:])
                nc.vector.tensor_scalar_mul(
                    hsq[:, cc * RT_PER_CH:(cc + 1) * RT_PER_CH], sl, 0.5)
                tps = tp_tile(f"tps_{cc}")
                t16 = tps[:].bitcast(FP16)[:, 0:P]
                nc.tensor.transpose(t16[:3 * RT_PER_CH, :], s3c[:], ident[:])
                t32 = sq3s.tile([3 * RT_PER_CH, P], FP16, tag="t32",
                                name=f"t32_{cc}")
                nc.vector.tensor_copy(t32[:], t16[:3 * RT_PER_CH, :])
                scr = dram.tile([3 * RT_PER_CH, P], FP16, tag="scr",
                                name=f"scr_{cc}")
                nc.sync.dma_start(scr[:], t32[:])
                for lv in range(3):
                    dst = sq3row[lv:lv + 1,
                                 cc * CHUNK:(cc + 1) * CHUNK].rearrange(
                        "a (m p) -> a m p", m=RT_PER_CH, p=P)
                    src = scr[lv * RT_PER_CH:(lv + 1) * RT_PER_CH,
                              :].unsqueeze(0)
                    nc.sync.dma_start(dst, src)

            # ---- main loop ----
            cands = {}
            saved_tiles = {}

            def rsl(xt, c):
                return xt[:, c * CHUNK:(c + 1) * CHUNK]

            def get_cand(r):
                if r not in cands:
                    cands[r] = candp.tile([P, 64], FP32, tag=f"cand{r}",
                                          name=f"cand_{r}")
                return cands[r]

            for m in range(N_RT):
                c0 = m // 4
                cs_all = list(range(c0, N_CH))
                wh0 = xt_h0[:, m * P:(m + 1) * P]
                wh1 = xt_h1[:, m * P:(m + 1) * P]
                wl0 = xt_l0[:, m * P:(m + 1) * P]
                wl1 = xt_l1[:, m * P:(m + 1) * P]
                cand = get_cand(m)
                for hs in range(0, len(cs_all), 4):
                    cs = cs_all[hs:hs + 4]
                    ps = {}
                    for c in cs:
                        ps[c] = mm.tile([P, CHUNK], FP32, tag="mmps",
                                        name=f"ps_{m}_{c}")
                    for c in cs:
                        nc.tensor.matmul(ps[c][:], wh0, rsl(xt_h0, c),
                                         start=True, stop=False)
                    for c in cs:
                        nc.tensor.matmul(ps[c][:], wh0, rsl(xt_l0, c),
                                         start=False, stop=False)
                    for c in cs:
                        nc.tensor.matmul(ps[c][:], wh1, rsl(xt_h1, c),
                                         start=False, stop=False)
                    for c in cs:
                        nc.tensor.matmul(ps[c][:], wh1, rsl(xt_l1, c),
                                         start=False, stop=False)
                    for c in cs:
                        nc.tensor.matmul(ps[c][:], wl0, rsl(xt_h0, c),
                                         start=False, stop=False)
                    for c in cs:
                        nc.tensor.matmul(ps[c][:], wl1, rsl(xt_h1, c),
                                         start=False, stop=False)
                    for c in cs:
                        nc.tensor.matmul(ps[c][:], ones3[:], rsl(sq3row, c),
                                         start=False, stop=True)
                    for c in cs:
                        nc.vector.max(cand[:, 8 * (c - c0):8 * (c - c0) + 8],
                                      ps[c][:])
                    for c in cs:
                        if c > c0:
                            sv = saved.tile([P, CHUNK], FP32, tag="sv",
                                            name=f"sv_{m}_{c}")
                            nc.scalar.copy(sv[:], ps[c][:])
                            saved_tiles[(m, c)] = sv

                # mirrors sourced from column-block group g = m//4
                if m % 4 == 3:
                    g = m // 4
                    js = [4 * g, 4 * g + 1, 4 * g + 2, 4 * g + 3]
                    for c in range(g + 1, N_CH):
                        for ri in range(RT_PER_CH):
                            r = 4 * c + ri
                            mp = tp_tile(f"mp_{r}_{g}")
                            nc.tensor.matmul(
                                mp[:], ones3[:],
                                sq3row[:, 4 * g * P:(4 * g + 4) * P],
                                start=True, stop=False, skip_group_check=True)
                            for ji, j in enumerate(js):
                                sv = saved_tiles[(j, c)]
                                nc.tensor.matmul(
                                    mp[:, ji * P:(ji + 1) * P],
                                    sv[:, ri * P:(ri + 1) * P], identf[:],
                                    is_transpose=True, start=False,
                                    stop=(ji == 3), skip_group_check=True)
                            # +sq_r/2 commutes with max8: add it to the
                            # 8-wide output instead of the 512-wide tile
                            slot = (N_CH - c) + g
                            rc = get_cand(r)
                            m8 = work.tile([P, 8], FP32, tag="m8",
                                           name=f"m8_{r}_{g}")
                            nc.vector.max(m8[:], mp[:])
                            nc.vector.tensor_scalar_add(
                                rc[:, 8 * slot:8 * slot + 8], m8[:],
                                hsq[:, r:r + 1])

                negs8 = work.tile([P, 8], FP32, tag="negs8", name=f"negs8_{m}")
                nc.vector.max(negs8[:], cand[:])
                nc.vector.scalar_tensor_tensor(dk2_all[:, m:m + 1],
                                               negs8[:, 5:6], -2.0,
                                               sq_all[:, m:m + 1],
                                               ALU.mult, ALU.add)

            nc.sync.dma_start(dk2_out, dk2_all[:])
    nc.compile()
    return nc


def _dk2_on_device(x_np):
    from concourse.bass_utils import run_bass_kernel_spmd
    global _NC
    if _NC is None:
        _NC = _build()
    in_maps = [{"x": np.ascontiguousarray(x_np[b])} for b in range(B)]
    res = run_bass_kernel_spmd(_NC, in_maps, core_ids=list(range(B)))
    # dk2[p, m] holds token 128*m + p
    return np.stack([res.results[b]["dk2"].T.reshape(-1) for b in range(B)])


def kernel(x):
    import jax
    import jax.numpy as jnp

    x_np = np.ascontiguousarray(np.asarray(x, dtype=np.float32))
    dk2 = _dk2_on_device(x_np)

    # host tail on the default jax device — same placement as the reference,
    # so the Gumbel PRNG bits and top_k semantics match bit-for-bit
    w = jnp.sqrt(jnp.asarray(np.maximum(dk2, 0.0), dtype=jnp.float32) + EPS)
    g = jax.random.gumbel(jax.random.key(42), w.shape, dtype=w.dtype)
    scores = jnp.log(w + EPS) + g
    _, idx = jax.lax.top_k(scores, SAMPLE_K)
    idx = np.asarray(idx)
    sampled = np.take_along_axis(x_np, idx[:, :, None], axis=1)
    return (sampled, 0.0)
